# revision 13
# baseline (speedup 1.0000x reference)
"""Bass/Tile Trainium2 kernel for batched self-attention with diagonal
self-exclusion (LSA): out = softmax(mask_diag(Q K^T / t)) @ V.

Shapes: Q,K,V [64, 1024, 768] fp32, temperature [1] fp32.
Sharding: batch dim across 8 NeuronCores (8 batches/core, pure data parallel).

Per-core algorithm (per batch b):
  - gpsimd cast-load Q,K,V fp32 HBM -> bf16 SBUF (natural [n, d] layout).
  - xbar DMA-transpose Q,K bf16 to d-major QT,KT [d, n] (128x128 blocks).
  - S^T[k, q] = sum_d KT[d,k] * QT[d,q] on PE (bf16, fp32 PSUM accum),
    k on partitions / q on free, 8 k-tiles x 2 q-halves x 6 d-chunks.
  - E = exp(S^T * (1/t)) on ScalarE (PSUM -> SBUF bf16), 1/t from input.
  - diagonal exclusion: E diag block *= (1 - I) mask.
  - out_psum[q, 0:769] = sum_k E^T[k,q] * [V | ones][k, :] on PE; col 768
    is the softmax denominator (ones-column trick).
  - out = out_psum[:, 0:768] * reciprocal(out_psum[:, 768]) -> bf16 SBUF
    -> HBM bf16 (host widens back to fp32; adds ~0.2% rounding, well
    inside the 2e-2 gate).

Engine roles are kept disjoint so no PE-feeding work ever queues behind a
long-blocking wait (the previous version lost ~90us to EXPs stuck behind
store DMAs on the Scalar queue):
  - GpSimd (SWDGE): input cast-loads only.
  - Sync (HWDGE):   xbar transposes + output stores (these two serialize
    against each other at HW level anyway).
  - Scalar:         EXP activations only.
  - Vector:         diag mask, reciprocal, output scale (bf16 out).
"""

import os
import sys

if "/opt/trn_rl_repo" not in sys.path:
    sys.path.insert(0, "/opt/trn_rl_repo")

import numpy as np
import ml_dtypes

import concourse.bass as bass
import concourse.bacc as bacc
import concourse.tile as tile
from concourse import mybir
from concourse.bass_utils import run_bass_kernel_spmd

B, N, D = 64, 1024, 768
NCORES = 8
BPC = B // NCORES  # batches per core
P = 128
NT = N // P   # 8 n-tiles (also k-tiles / q-tiles)
DJ = D // P   # 6 d-chunks
F32 = mybir.dt.float32
BF16 = mybir.dt.bfloat16


def build_program(bpc: int = BPC) -> bacc.Bacc:
    nc = bacc.Bacc(
        "TRN2",
        target_bir_lowering=False,
        debug=False,
        num_devices=NCORES,
        num_swdge_queues=4,
    )
    q_h = nc.dram_tensor("q", [bpc, N, D], F32, kind="ExternalInput").ap()
    k_h = nc.dram_tensor("k", [bpc, N, D], F32, kind="ExternalInput").ap()
    v_h = nc.dram_tensor("v", [bpc, N, D], F32, kind="ExternalInput").ap()
    t_h = nc.dram_tensor("t", [1], F32, kind="ExternalInput").ap()
    m_h = nc.dram_tensor("mask", [P, P], BF16, kind="ExternalInput").ap()
    o_h = nc.dram_tensor("o", [bpc, N, D], BF16, kind="ExternalOutput").ap()

    with tile.TileContext(nc) as tc:
        # Dependency tracking for DMA instructions (loads, stores, xbar
        # transposes) is TAG-level coarse: their ISA-lowered access
        # patterns lose tile identity, so a multi-buf ring on one tag
        # creates false WAR edges (e.g. batch b+1's transpose waiting on
        # ALL of batch b's S^T matmuls because both touch the same tag).
        # Tags touched by DMAs are therefore rotated per batch (bufs=1
        # per tag) instead of multi-buffered.
        with (
            tc.tile_pool(name="const", bufs=1) as const,
            tc.tile_pool(name="stage", bufs=1) as stage,
            tc.tile_pool(name="vpool", bufs=1) as vpool,
            tc.tile_pool(name="tpose", bufs=1) as tpose,
            tc.tile_pool(name="epool", bufs=2) as epool,
            tc.tile_pool(name="opool", bufs=1) as opool,
            tc.tile_pool(name="small", bufs=8) as small,
            tc.tile_pool(name="ps_s", bufs=4, space="PSUM") as ps_s,
            tc.tile_pool(name="ps_o", bufs=2, space="PSUM") as ps_o,
        ):
            # constants: 1/temperature broadcast to all partitions, diag mask
            t_bc = const.tile([P, 1], F32)
            nc.gpsimd.dma_start(out=t_bc, in_=t_h.to_broadcast((P, 1)))
            inv_t = const.tile([P, 1], F32)
            nc.vector.reciprocal(inv_t, t_bc)
            mask_sb = const.tile([P, P], BF16)
            nc.sync.dma_start(out=mask_sb, in_=m_h)

            def load_and_transpose(b):
                """Batch b's input chain. K/Q: gpsimd cast-loads + xbar
                transposes on sync. These four ops form a serial chain (a
                HW-deadlock guard serializes SWDGE DMAs against in-flight
                xbar transposes, in issue order), so everything else stays
                OFF that chain: V is loaded fp32 in two halves via HWDGE
                (sync queue, async) and cast to bf16 on the Vector engine.
                Chain length ~32us/batch < the PE's 41us/batch.
                Returns (qkT, v_sb); qT = qkT[:, 0], kT = qkT[:, 1]."""
                kst = stage.tile([P, NT, D], BF16, tag=f"sk{b % 2}")
                qst = stage.tile([P, NT, D], BF16, tag=f"sq{b % 2}")
                # xbar 3D-out semantics: out[p, j, r] = in[r, j*128 + p] with
                # j = (nt, dj) merged: qkT[p,t,nt,dj,r] = QK[t][nt*128+r, dj*128+p]
                qkT = tpose.tile([P, 2, NT, DJ, P], BF16, tag=f"t{b % 2}")
                H = NT // 2
                if b == 0:
                    # Half granules, load->transpose interleaved (the guard
                    # makes the chain serial anyway); first matmul ~16us in.
                    h0, h1 = slice(0, H), slice(H, NT)
                    r0, r1 = slice(0, H * P), slice(H * P, N)
                    for st, t, src, rows, nts in (
                        (kst, 1, k_h, r0, h0),
                        (qst, 0, q_h, r0, h0),
                        (qst, 0, q_h, r1, h1),
                        (kst, 1, k_h, r1, h1),
                    ):
                        nc.gpsimd.dma_start(
                            out=st[:, nts, :],
                            in_=src[b, rows, :].rearrange(
                                "(nt p) d -> p nt d", p=P
                            ),
                        )
                        nc.sync.dma_start(
                            out=qkT[:, t, nts, :, :],
                            in_=st[:, nts, :],
                            transpose=True,
                        )
                else:
                    nc.gpsimd.dma_start(
                        out=kst,
                        in_=k_h[b].rearrange("(nt p) d -> p nt d", p=P),
                    )
                    nc.sync.dma_start(
                        out=qkT[:, 1], in_=kst, transpose=True
                    )
                    nc.gpsimd.dma_start(
                        out=qst,
                        in_=q_h[b].rearrange("(nt p) d -> p nt d", p=P),
                    )
                    nc.sync.dma_start(
                        out=qkT[:, 0], in_=qst, transpose=True
                    )
                # V: fp32 half-loads on the HWDGE sync queue (async issue,
                # not part of the guarded SWDGE<->xbar chain), cast to bf16
                # by the Vector engine.
                v32 = stage.tile([P, H, D], F32, tag=f"v32{b % 2}")
                v_sb = vpool.tile([P, NT, D + 1], BF16, tag=f"v{b % 2}")
                for h in range(2):
                    rows = slice(h * H * P, (h + 1) * H * P)
                    nc.sync.dma_start(
                        out=v32,
                        in_=v_h[b, rows, :].rearrange("(nt p) d -> p nt d", p=P),
                    )
                    nc.vector.tensor_copy(
                        v_sb[:, h * H : (h + 1) * H, 0:D], v32
                    )
                nc.vector.memset(v_sb[:, :, D : D + 1], 1.0)
                return qkT, v_sb

            # 1-deep software pipeline: batch b+1's DMA chain is issued
            # before batch b's compute in program order, so the DMA engines
            # stay packed while the PE works on batch b.
            pending = load_and_transpose(0)
            for b in range(bpc):
                qkT, v_sb = pending
                if b + 1 < bpc:
                    pending = load_and_transpose(b + 1)

                # ---- S^T = K Q^T (k on partitions), exp, diag-mask
                ev = epool.tile([P, NT, N], BF16, tag="ev")
                for kh in range(2):
                    for half in range(2):
                        for kt in range(4 * kh, 4 * kh + 4):
                            sT = ps_s.tile([P, 512], F32, tag="sT")
                            for dj in range(DJ):
                                nc.tensor.matmul(
                                    sT,
                                    lhsT=qkT[:, 1, kt, dj, :],
                                    rhs=qkT[:, 0, 4 * half : 4 * half + 4, dj, :],
                                    start=(dj == 0),
                                    stop=(dj == DJ - 1),
                                )
                            nc.scalar.activation(
                                ev[:, kt, half * 512 : half * 512 + 512],
                                sT,
                                mybir.ActivationFunctionType.Exp,
                                scale=inv_t,
                            )
                            if kt // 4 == half:
                                nc.vector.tensor_mul(
                                    ev[:, kt, kt * P : (kt + 1) * P],
                                    ev[:, kt, kt * P : (kt + 1) * P],
                                    mask_sb,
                                )

                # ---- out = (E^T @ [V | 1]) then normalize by ones-column.
                # Outputs staged four q-tiles per store (786 KB bf16 DMAs)
                # and shipped on the sync queue (never in front of EXPs).
                o_sb = None
                for qt in range(NT):
                    o_ps = ps_o.tile([P, D + 1], F32, tag="o_ps")
                    for kt in range(NT):
                        nc.tensor.matmul(
                            o_ps[:, 0:512],
                            lhsT=ev[:, kt, qt * P : (qt + 1) * P],
                            rhs=v_sb[:, kt, 0:512],
                            start=(kt == 0),
                            stop=(kt == NT - 1),
                        )
                    for kt in range(NT):
                        nc.tensor.matmul(
                            o_ps[:, 512 : D + 1],
                            lhsT=ev[:, kt, qt * P : (qt + 1) * P],
                            rhs=v_sb[:, kt, 512 : D + 1],
                            start=(kt == 0),
                            stop=(kt == NT - 1),
                        )
                    rs = small.tile([P, 1], F32, tag="rs")
                    nc.vector.reciprocal(rs, o_ps[:, D : D + 1])
                    if qt % 4 == 0:
                        o_sb = opool.tile(
                            [P, 4, D], BF16, tag=f"o{(2 * b + qt // 4) % 3}"
                        )
                    nc.vector.tensor_scalar_mul(
                        o_sb[:, qt % 4, :], o_ps[:, 0:D], rs
                    )
                    if qt % 4 == 3:
                        nc.scalar.dma_start(
                            out=o_h[b, (qt - 3) * P : (qt + 1) * P, :].rearrange(
                                "(j p) d -> p j d", p=P
                            ),
                            in_=o_sb,
                        )
    nc.finalize()
    return nc


_prog_cache: dict[int, bacc.Bacc] = {}


def _get_program(bpc: int) -> bacc.Bacc:
    if bpc not in _prog_cache:
        _prog_cache[bpc] = build_program(bpc)
    return _prog_cache[bpc]


def _run(Q, K, V, temperature, bpc: int = BPC, trace: bool = False):
    nc = _get_program(bpc)
    mask = (1.0 - np.eye(P, dtype=np.float32)).astype(ml_dtypes.bfloat16)
    t = np.asarray(temperature, dtype=np.float32).reshape(1)
    in_maps = []
    for c in range(NCORES):
        sl = slice(c * bpc, (c + 1) * bpc)
        in_maps.append(
            {
                "q": np.ascontiguousarray(Q[sl], dtype=np.float32),
                "k": np.ascontiguousarray(K[sl], dtype=np.float32),
                "v": np.ascontiguousarray(V[sl], dtype=np.float32),
                "t": t,
                "mask": mask,
            }
        )
    res = run_bass_kernel_spmd(
        nc, in_maps, core_ids=list(range(NCORES)), trace=trace
    )
    out = np.concatenate([r["o"] for r in res.results], axis=0)
    return out, res


def kernel(Q, K, V, temperature):
    # If BASS_TRACE leaked into the environment, the trace path would need
    # antenv.axon_hooks (absent in this image) and crash; force it off for
    # the plain grading path.
    if os.environ.get("BASS_TRACE"):
        try:
            import antenv.axon_hooks  # noqa: F401
        except ImportError:
            os.environ.pop("BASS_TRACE", None)
    out, _ = _run(Q, K, V, temperature)
    return np.asarray(out).astype(np.float32)


# revision 15
# speedup vs baseline: 1.0786x; 1.0786x over previous
"""Bass/Tile Trainium2 kernel for batched self-attention with diagonal
self-exclusion (LSA): out = softmax(mask_diag(Q K^T / t)) @ V.

Shapes: Q,K,V [64, 1024, 768] fp32, temperature [1] fp32.
Sharding: batch dim across 8 NeuronCores (8 batches/core, pure data parallel).

Per-core algorithm (per batch b):
  - gpsimd cast-load Q,K,V fp32 HBM -> bf16 SBUF (natural [n, d] layout).
  - xbar DMA-transpose Q,K bf16 to d-major QT,KT [d, n] (128x128 blocks).
  - S^T[k, q] = sum_d KT[d,k] * QT[d,q] on PE (bf16, fp32 PSUM accum),
    k on partitions / q on free, 8 k-tiles x 2 q-halves x 6 d-chunks.
  - E = exp(S^T * (1/t)) on ScalarE (PSUM -> SBUF bf16), 1/t from input.
  - diagonal exclusion: E diag block *= (1 - I) mask.
  - out_psum[q, 0:769] = sum_k E^T[k,q] * [V | ones][k, :] on PE; col 768
    is the softmax denominator (ones-column trick).
  - out = out_psum[:, 0:768] * reciprocal(out_psum[:, 768]) -> bf16 SBUF
    -> HBM bf16 (host widens back to fp32; adds ~0.2% rounding, well
    inside the 2e-2 gate).

Engine roles are kept disjoint so no PE-feeding work ever queues behind a
long-blocking wait (the previous version lost ~90us to EXPs stuck behind
store DMAs on the Scalar queue):
  - GpSimd (SWDGE): input cast-loads only.
  - Sync (HWDGE):   xbar transposes + output stores (these two serialize
    against each other at HW level anyway).
  - Scalar:         EXP activations only.
  - Vector:         diag mask, reciprocal, output scale (bf16 out).
"""

import os
import sys

if "/opt/trn_rl_repo" not in sys.path:
    sys.path.insert(0, "/opt/trn_rl_repo")

import numpy as np
import ml_dtypes

import concourse.bass as bass
import concourse.bacc as bacc
import concourse.tile as tile
from concourse import mybir
from concourse.bass_utils import run_bass_kernel_spmd

B, N, D = 64, 1024, 768
NCORES = 8
BPC = B // NCORES  # batches per core
P = 128
NT = N // P   # 8 n-tiles (also k-tiles / q-tiles)
DJ = D // P   # 6 d-chunks
F32 = mybir.dt.float32
BF16 = mybir.dt.bfloat16


def build_program(bpc: int = BPC) -> bacc.Bacc:
    nc = bacc.Bacc(
        "TRN2",
        target_bir_lowering=False,
        debug=False,
        num_devices=NCORES,
        num_swdge_queues=4,
    )
    q_h = nc.dram_tensor("q", [bpc, N, D], F32, kind="ExternalInput").ap()
    k_h = nc.dram_tensor("k", [bpc, N, D], F32, kind="ExternalInput").ap()
    v_h = nc.dram_tensor("v", [bpc, N, D], F32, kind="ExternalInput").ap()
    t_h = nc.dram_tensor("t", [1], F32, kind="ExternalInput").ap()
    m_h = nc.dram_tensor("mask", [P, P], BF16, kind="ExternalInput").ap()
    o_h = nc.dram_tensor("o", [bpc, N, D], BF16, kind="ExternalOutput").ap()

    with tile.TileContext(nc) as tc:
        # Dependency tracking for DMA instructions (loads, stores, xbar
        # transposes) is TAG-level coarse: their ISA-lowered access
        # patterns lose tile identity, so a multi-buf ring on one tag
        # creates false WAR edges (e.g. batch b+1's transpose waiting on
        # ALL of batch b's S^T matmuls because both touch the same tag).
        # Tags touched by DMAs are therefore rotated per batch (bufs=1
        # per tag) instead of multi-buffered.
        with (
            tc.tile_pool(name="const", bufs=1) as const,
            tc.tile_pool(name="stage", bufs=1) as stage,
            tc.tile_pool(name="vpool", bufs=1) as vpool,
            tc.tile_pool(name="tpose", bufs=1) as tpose,
            tc.tile_pool(name="epool", bufs=2) as epool,
            tc.tile_pool(name="opool", bufs=1) as opool,
            tc.tile_pool(name="small", bufs=8) as small,
            tc.tile_pool(name="ps_s", bufs=4, space="PSUM") as ps_s,
            tc.tile_pool(name="ps_o", bufs=2, space="PSUM") as ps_o,
        ):
            # constants: 1/temperature broadcast to all partitions, diag mask
            t_bc = const.tile([P, 1], F32)
            nc.gpsimd.dma_start(out=t_bc, in_=t_h.to_broadcast((P, 1)))
            inv_t = const.tile([P, 1], F32)
            nc.vector.reciprocal(inv_t, t_bc)
            mask_sb = const.tile([P, P], BF16)
            nc.sync.dma_start(out=mask_sb, in_=m_h)

            def load_and_transpose(b):
                """Batch b's input chain. K/Q: gpsimd cast-loads + xbar
                transposes on sync. These four ops form a serial chain (a
                HW-deadlock guard serializes SWDGE DMAs against in-flight
                xbar transposes, in issue order), so everything else stays
                OFF that chain: V is loaded fp32 in two halves via HWDGE
                (sync queue, async) and cast to bf16 on the Vector engine.
                Chain length ~32us/batch < the PE's 41us/batch.
                Returns (qkT, v_sb); qT = qkT[:, 0], kT = qkT[:, 1]."""
                kst = stage.tile([P, NT, D], BF16, tag=f"sk{b % 2}")
                qst = stage.tile([P, NT, D], BF16, tag=f"sq{b % 2}")
                # xbar 3D-out semantics: out[p, j, r] = in[r, j*128 + p] with
                # j = (nt, dj) merged: qkT[p,t,nt,dj,r] = QK[t][nt*128+r, dj*128+p]
                qkT = tpose.tile([P, 2, NT, DJ, P], BF16, tag=f"t{b % 2}")
                H = NT // 2
                if b == 0:
                    # Half granules: all loads first, then the transposes.
                    h0, h1 = slice(0, H), slice(H, NT)
                    r0, r1 = slice(0, H * P), slice(H * P, N)
                    granules = (
                        (kst, 1, k_h, r0, h0),
                        (qst, 0, q_h, r0, h0),
                        (qst, 0, q_h, r1, h1),
                        (kst, 1, k_h, r1, h1),
                    )
                    for st, t, src, rows, nts in granules:
                        nc.gpsimd.dma_start(
                            out=st[:, nts, :],
                            in_=src[b, rows, :].rearrange(
                                "(nt p) d -> p nt d", p=P
                            ),
                        )
                    for st, t, src, rows, nts in granules:
                        nc.sync.dma_start(
                            out=qkT[:, t, nts, :, :],
                            in_=st[:, nts, :],
                            transpose=True,
                        )
                else:
                    nc.gpsimd.dma_start(
                        out=kst,
                        in_=k_h[b].rearrange("(nt p) d -> p nt d", p=P),
                    )
                    nc.sync.dma_start(
                        out=qkT[:, 1], in_=kst, transpose=True
                    )
                    nc.gpsimd.dma_start(
                        out=qst,
                        in_=q_h[b].rearrange("(nt p) d -> p nt d", p=P),
                    )
                    nc.sync.dma_start(
                        out=qkT[:, 0], in_=qst, transpose=True
                    )
                # V: fp32 half-loads on the HWDGE scalar queue (async issue,
                # away from both the guarded SWDGE<->xbar chain and the sync
                # queue's guard waits), cast to bf16 by the Vector engine.
                # Two staging tags so the second issue never blocks the
                # scalar engine waiting on the first half's cast.
                v_sb = vpool.tile([P, NT, D + 1], BF16, tag=f"v{b % 2}")
                for h in range(2):
                    v32 = stage.tile([P, H, D], F32, tag=f"v32{h}")
                    rows = slice(h * H * P, (h + 1) * H * P)
                    nc.scalar.dma_start(
                        out=v32,
                        in_=v_h[b, rows, :].rearrange("(nt p) d -> p nt d", p=P),
                    )
                    nc.vector.tensor_copy(
                        v_sb[:, h * H : (h + 1) * H, 0:D], v32
                    )
                nc.vector.memset(v_sb[:, :, D : D + 1], 1.0)
                return qkT, v_sb

            # 1-deep software pipeline: batch b+1's DMA chain is issued
            # before batch b's compute in program order, so the DMA engines
            # stay packed while the PE works on batch b.
            pending = load_and_transpose(0)
            for b in range(bpc):
                qkT, v_sb = pending
                if b + 1 < bpc:
                    pending = load_and_transpose(b + 1)

                # ---- S^T = K Q^T (k on partitions), exp, diag-mask
                ev = epool.tile([P, NT, N], BF16, tag="ev")
                for kh in range(2):
                    for half in range(2):
                        for kt in range(4 * kh, 4 * kh + 4):
                            sT = ps_s.tile([P, 512], F32, tag="sT")
                            for dj in range(DJ):
                                nc.tensor.matmul(
                                    sT,
                                    lhsT=qkT[:, 1, kt, dj, :],
                                    rhs=qkT[:, 0, 4 * half : 4 * half + 4, dj, :],
                                    start=(dj == 0),
                                    stop=(dj == DJ - 1),
                                )
                            nc.scalar.activation(
                                ev[:, kt, half * 512 : half * 512 + 512],
                                sT,
                                mybir.ActivationFunctionType.Exp,
                                scale=inv_t,
                            )
                            if kt // 4 == half:
                                nc.vector.tensor_mul(
                                    ev[:, kt, kt * P : (kt + 1) * P],
                                    ev[:, kt, kt * P : (kt + 1) * P],
                                    mask_sb,
                                )

                # ---- out = (E^T @ [V | 1]) then normalize by ones-column.
                # Outputs staged four q-tiles per store (786 KB bf16 DMAs)
                # and shipped on the sync queue (never in front of EXPs).
                o_sb = None
                for qt in range(NT):
                    o_ps = ps_o.tile([P, D + 1], F32, tag="o_ps")
                    for kt in range(NT):
                        nc.tensor.matmul(
                            o_ps[:, 0:512],
                            lhsT=ev[:, kt, qt * P : (qt + 1) * P],
                            rhs=v_sb[:, kt, 0:512],
                            start=(kt == 0),
                            stop=(kt == NT - 1),
                        )
                    for kt in range(NT):
                        nc.tensor.matmul(
                            o_ps[:, 512 : D + 1],
                            lhsT=ev[:, kt, qt * P : (qt + 1) * P],
                            rhs=v_sb[:, kt, 512 : D + 1],
                            start=(kt == 0),
                            stop=(kt == NT - 1),
                        )
                    rs = small.tile([P, 1], F32, tag="rs")
                    nc.vector.reciprocal(rs, o_ps[:, D : D + 1])
                    if qt % 4 == 0:
                        o_sb = opool.tile(
                            [P, 4, D], BF16, tag=f"o{(2 * b + qt // 4) % 3}"
                        )
                    nc.vector.tensor_scalar_mul(
                        o_sb[:, qt % 4, :], o_ps[:, 0:D], rs
                    )
                    if qt % 4 == 3:
                        nc.scalar.dma_start(
                            out=o_h[b, (qt - 3) * P : (qt + 1) * P, :].rearrange(
                                "(j p) d -> p j d", p=P
                            ),
                            in_=o_sb,
                        )
    nc.finalize()
    return nc


_prog_cache: dict[int, bacc.Bacc] = {}


def _get_program(bpc: int) -> bacc.Bacc:
    if bpc not in _prog_cache:
        _prog_cache[bpc] = build_program(bpc)
    return _prog_cache[bpc]


def _run(Q, K, V, temperature, bpc: int = BPC, trace: bool = False):
    nc = _get_program(bpc)
    mask = (1.0 - np.eye(P, dtype=np.float32)).astype(ml_dtypes.bfloat16)
    t = np.asarray(temperature, dtype=np.float32).reshape(1)
    in_maps = []
    for c in range(NCORES):
        sl = slice(c * bpc, (c + 1) * bpc)
        in_maps.append(
            {
                "q": np.ascontiguousarray(Q[sl], dtype=np.float32),
                "k": np.ascontiguousarray(K[sl], dtype=np.float32),
                "v": np.ascontiguousarray(V[sl], dtype=np.float32),
                "t": t,
                "mask": mask,
            }
        )
    res = run_bass_kernel_spmd(
        nc, in_maps, core_ids=list(range(NCORES)), trace=trace
    )
    out = np.concatenate([r["o"] for r in res.results], axis=0)
    return out, res


def kernel(Q, K, V, temperature):
    # If BASS_TRACE leaked into the environment, the trace path would need
    # antenv.axon_hooks (absent in this image) and crash; force it off for
    # the plain grading path.
    if os.environ.get("BASS_TRACE"):
        try:
            import antenv.axon_hooks  # noqa: F401
        except ImportError:
            os.environ.pop("BASS_TRACE", None)
    out, _ = _run(Q, K, V, temperature)
    return np.asarray(out).astype(np.float32)


# revision 16
# speedup vs baseline: 1.0856x; 1.0065x over previous
"""Bass/Tile Trainium2 kernel for batched self-attention with diagonal
self-exclusion (LSA): out = softmax(mask_diag(Q K^T / t)) @ V.

Shapes: Q,K,V [64, 1024, 768] fp32, temperature [1] fp32.
Sharding: batch dim across 8 NeuronCores (8 batches/core, pure data parallel).

Per-core algorithm (per batch b):
  - K: gpsimd cast-load fp32->bf16, then xbar DMA-transpose to d-major
    KT [d, n]. These two ops form a serial chain with each other and any
    other SWDGE DMA (a HW-deadlock guard serializes SWDGE DMAs against
    in-flight xbar transposes in issue order), so K is the ONLY tensor on
    that chain: ~27us/batch, comfortably under the PE's 41us/batch.
  - Q: gpsimd cast-load (natural layout), then transposed 128x128-tile-wise
    on the PE (transpose-mode matmul via identity, ~6us/batch) into PSUM,
    copied to SBUF by the Vector engine. Off the DMA chain entirely.
  - V: fp32 half-loads on the HWDGE scalar queue (async issue), cast to
    bf16 by the Vector engine; a ones-column is appended.
  - S^T[k, q] = sum_d KT[d,k] * QT[d,q] on PE (bf16, fp32 PSUM accum),
    k on partitions / q on free, 8 k-tiles x 2 q-halves x 6 d-chunks.
  - E = exp(S^T * (1/t)) on ScalarE (PSUM -> SBUF bf16), 1/t from input.
  - diagonal exclusion: E diag block *= (1 - I) mask.
  - out_psum[q, 0:769] = sum_k E^T[k,q] * [V | ones][k, :] on PE; col 768
    is the softmax denominator (ones-column trick).
  - out = out_psum[:, 0:768] * reciprocal(out_psum[:, 768]) -> bf16 SBUF
    -> HBM bf16 via scalar queue (host widens to fp32; ~0.2% rounding,
    well inside the 2e-2 gate).

Engine roles: GpSimd = K/Q cast-loads only. Sync = K transposes only.
Scalar = EXPs + V loads + output stores (all async or prompt). Vector =
V casts, QT copies, diag mask, reciprocal, output scale. Tags touched by
DMAs rotate per batch (DMA dep tracking is tag-coarse; ring reuse on one
tag creates false WAR edges).
"""

import os
import sys

if "/opt/trn_rl_repo" not in sys.path:
    sys.path.insert(0, "/opt/trn_rl_repo")

import numpy as np
import ml_dtypes

import concourse.bass as bass
import concourse.bacc as bacc
import concourse.tile as tile
from concourse import mybir
from concourse.bass_utils import run_bass_kernel_spmd

B, N, D = 64, 1024, 768
NCORES = 8
BPC = B // NCORES  # batches per core
P = 128
NT = N // P   # 8 n-tiles (also k-tiles / q-tiles)
DJ = D // P   # 6 d-chunks
H = NT // 2
F32 = mybir.dt.float32
BF16 = mybir.dt.bfloat16


def build_program(bpc: int = BPC) -> bacc.Bacc:
    nc = bacc.Bacc(
        "TRN2",
        target_bir_lowering=False,
        debug=False,
        num_devices=NCORES,
        num_swdge_queues=4,
    )
    q_h = nc.dram_tensor("q", [bpc, N, D], F32, kind="ExternalInput").ap()
    k_h = nc.dram_tensor("k", [bpc, N, D], F32, kind="ExternalInput").ap()
    v_h = nc.dram_tensor("v", [bpc, N, D], F32, kind="ExternalInput").ap()
    t_h = nc.dram_tensor("t", [1], F32, kind="ExternalInput").ap()
    m_h = nc.dram_tensor("mask", [P, P], BF16, kind="ExternalInput").ap()
    i_h = nc.dram_tensor("ident", [P, P], BF16, kind="ExternalInput").ap()
    o_h = nc.dram_tensor("o", [bpc, N, D], BF16, kind="ExternalOutput").ap()

    with tile.TileContext(nc) as tc:
        with (
            tc.tile_pool(name="const", bufs=1) as const,
            tc.tile_pool(name="stage", bufs=1) as stage,
            tc.tile_pool(name="vpool", bufs=1) as vpool,
            tc.tile_pool(name="tpose", bufs=1) as tpose,
            tc.tile_pool(name="epool", bufs=2) as epool,
            tc.tile_pool(name="opool", bufs=1) as opool,
            tc.tile_pool(name="small", bufs=8) as small,
            tc.tile_pool(name="ps_s", bufs=2, space="PSUM") as ps_s,
            tc.tile_pool(name="ps_o", bufs=2, space="PSUM") as ps_o,
            tc.tile_pool(name="ps_t", bufs=2, space="PSUM") as ps_t,
        ):
            # constants: 1/temperature broadcast, diag mask, identity
            t_bc = const.tile([P, 1], F32)
            nc.gpsimd.dma_start(out=t_bc, in_=t_h.to_broadcast((P, 1)))
            inv_t = const.tile([P, 1], F32)
            nc.vector.reciprocal(inv_t, t_bc)
            mask_sb = const.tile([P, P], BF16)
            nc.sync.dma_start(out=mask_sb, in_=m_h)
            ident = const.tile([P, P], BF16)
            nc.sync.dma_start(out=ident, in_=i_h)

            def load_batch(b):
                """Issue batch b's loads and K's xbar transpose.
                Returns (kT, qst, qT, v_sb); qT is filled later by
                emit_q_transposes."""
                kst = stage.tile([P, NT, D], BF16, tag=f"sk{b % 2}")
                qst = stage.tile([P, NT, D], BF16, tag=f"sq{b % 2}")
                # xbar 3D-out semantics: out[p, j, r] = in[r, j*128 + p],
                # j = (nt, dj) merged: kT[p,nt,dj,r] = K[nt*128+r, dj*128+p]
                kT = tpose.tile([P, NT, DJ, P], BF16, tag=f"tk{b % 2}")
                qT = tpose.tile([P, NT, DJ, P], BF16, tag=f"tq{b % 2}")
                if b == 0:
                    # Q first (feeds the PE transposes while K's chain
                    # runs), K in halves so its first transpose lands early.
                    nc.gpsimd.dma_start(
                        out=qst,
                        in_=q_h[b].rearrange("(nt p) d -> p nt d", p=P),
                    )
                    for h in range(2):
                        nts = slice(h * H, (h + 1) * H)
                        rows = slice(h * H * P, (h + 1) * H * P)
                        nc.gpsimd.dma_start(
                            out=kst[:, nts, :],
                            in_=k_h[b, rows, :].rearrange(
                                "(nt p) d -> p nt d", p=P
                            ),
                        )
                        nc.sync.dma_start(
                            out=kT[:, nts, :, :],
                            in_=kst[:, nts, :],
                            transpose=True,
                        )
                else:
                    nc.gpsimd.dma_start(
                        out=kst,
                        in_=k_h[b].rearrange("(nt p) d -> p nt d", p=P),
                    )
                    nc.sync.dma_start(out=kT, in_=kst, transpose=True)
                    nc.gpsimd.dma_start(
                        out=qst,
                        in_=q_h[b].rearrange("(nt p) d -> p nt d", p=P),
                    )
                # V: fp32 half-loads on the scalar HWDGE queue, bf16 cast
                # on Vector. Two staging tags so the second issue never
                # blocks the scalar engine on the first half's cast.
                v_sb = vpool.tile([P, NT, D + 1], BF16, tag=f"v{b % 2}")
                for h in range(2):
                    v32 = stage.tile([P, H, D], F32, tag=f"v32{h}")
                    rows = slice(h * H * P, (h + 1) * H * P)
                    nc.scalar.dma_start(
                        out=v32,
                        in_=v_h[b, rows, :].rearrange("(nt p) d -> p nt d", p=P),
                    )
                    nc.vector.tensor_copy(
                        v_sb[:, h * H : (h + 1) * H, 0:D], v32
                    )
                nc.vector.memset(v_sb[:, :, D : D + 1], 1.0)
                return kT, qst, qT, v_sb

            def emit_q_transposes(batch):
                """PE transpose-mode: qst [q, d] -> qT [d, q], one PSUM
                bank-tile (6 dj blocks) per n-tile, drained to SBUF by
                the Vector engine."""
                _, qst, qT, _ = batch
                for nt in range(NT):
                    pt = ps_t.tile([P, DJ, P], BF16, tag="pt")
                    for dj in range(DJ):
                        nc.tensor.transpose(
                            pt[:, dj, :],
                            qst[:, nt, dj * P : (dj + 1) * P],
                            ident,
                        )
                    nc.vector.tensor_copy(qT[:, nt, :, :], pt)

            pending = load_batch(0)
            emit_q_transposes(pending)
            for b in range(bpc):
                kT, qst, qT, v_sb = pending
                if b + 1 < bpc:
                    pending = load_batch(b + 1)

                # ---- S^T = K Q^T (k on partitions), exp, diag-mask
                ev = epool.tile([P, NT, N], BF16, tag="ev")
                for kh in range(2):
                    for half in range(2):
                        for kt in range(4 * kh, 4 * kh + 4):
                            sT = ps_s.tile([P, 512], F32, tag="sT")
                            for dj in range(DJ):
                                nc.tensor.matmul(
                                    sT,
                                    lhsT=kT[:, kt, dj, :],
                                    rhs=qT[:, 4 * half : 4 * half + 4, dj, :],
                                    start=(dj == 0),
                                    stop=(dj == DJ - 1),
                                )
                            nc.scalar.activation(
                                ev[:, kt, half * 512 : half * 512 + 512],
                                sT,
                                mybir.ActivationFunctionType.Exp,
                                scale=inv_t,
                            )
                            if kt // 4 == half:
                                nc.vector.tensor_mul(
                                    ev[:, kt, kt * P : (kt + 1) * P],
                                    ev[:, kt, kt * P : (kt + 1) * P],
                                    mask_sb,
                                )

                # next batch's Q transposes ride the PE between S^T and AV
                if b + 1 < bpc:
                    emit_q_transposes(pending)

                # ---- out = (E^T @ [V | 1]) then normalize by ones-column.
                # Outputs staged four q-tiles per store (786 KB bf16 DMAs)
                # on the scalar queue.
                o_sb = None
                for qt in range(NT):
                    o_ps = ps_o.tile([P, D + 1], F32, tag="o_ps")
                    for kt in range(NT):
                        nc.tensor.matmul(
                            o_ps[:, 0:512],
                            lhsT=ev[:, kt, qt * P : (qt + 1) * P],
                            rhs=v_sb[:, kt, 0:512],
                            start=(kt == 0),
                            stop=(kt == NT - 1),
                        )
                    for kt in range(NT):
                        nc.tensor.matmul(
                            o_ps[:, 512 : D + 1],
                            lhsT=ev[:, kt, qt * P : (qt + 1) * P],
                            rhs=v_sb[:, kt, 512 : D + 1],
                            start=(kt == 0),
                            stop=(kt == NT - 1),
                        )
                    rs = small.tile([P, 1], F32, tag="rs")
                    nc.vector.reciprocal(rs, o_ps[:, D : D + 1])
                    if qt % 4 == 0:
                        o_sb = opool.tile(
                            [P, 4, D], BF16, tag=f"o{(2 * b + qt // 4) % 3}"
                        )
                    nc.vector.tensor_scalar_mul(
                        o_sb[:, qt % 4, :], o_ps[:, 0:D], rs
                    )
                    if qt % 4 == 3:
                        nc.scalar.dma_start(
                            out=o_h[b, (qt - 3) * P : (qt + 1) * P, :].rearrange(
                                "(j p) d -> p j d", p=P
                            ),
                            in_=o_sb,
                        )
    nc.finalize()
    return nc


_prog_cache: dict[int, bacc.Bacc] = {}


def _get_program(bpc: int) -> bacc.Bacc:
    if bpc not in _prog_cache:
        _prog_cache[bpc] = build_program(bpc)
    return _prog_cache[bpc]


def _run(Q, K, V, temperature, bpc: int = BPC, trace: bool = False):
    nc = _get_program(bpc)
    mask = (1.0 - np.eye(P, dtype=np.float32)).astype(ml_dtypes.bfloat16)
    ident = np.eye(P, dtype=np.float32).astype(ml_dtypes.bfloat16)
    t = np.asarray(temperature, dtype=np.float32).reshape(1)
    in_maps = []
    for c in range(NCORES):
        sl = slice(c * bpc, (c + 1) * bpc)
        in_maps.append(
            {
                "q": np.ascontiguousarray(Q[sl], dtype=np.float32),
                "k": np.ascontiguousarray(K[sl], dtype=np.float32),
                "v": np.ascontiguousarray(V[sl], dtype=np.float32),
                "t": t,
                "mask": mask,
                "ident": ident,
            }
        )
    res = run_bass_kernel_spmd(
        nc, in_maps, core_ids=list(range(NCORES)), trace=trace
    )
    out = np.concatenate([r["o"] for r in res.results], axis=0)
    return out, res


def kernel(Q, K, V, temperature):
    # If BASS_TRACE leaked into the environment, the trace path would need
    # antenv.axon_hooks (absent in this image) and crash; force it off for
    # the plain grading path.
    if os.environ.get("BASS_TRACE"):
        try:
            import antenv.axon_hooks  # noqa: F401
        except ImportError:
            os.environ.pop("BASS_TRACE", None)
    out, _ = _run(Q, K, V, temperature)
    return np.asarray(out).astype(np.float32)


# revision 20
# speedup vs baseline: 1.1044x; 1.0173x over previous
"""Bass/Tile Trainium2 kernel for batched self-attention with diagonal
self-exclusion (LSA): out = softmax(mask_diag(Q K^T / t)) @ V.

Shapes: Q,K,V [64, 1024, 768] fp32, temperature [1] fp32.
Sharding: batch dim across 8 NeuronCores (8 batches/core, pure data parallel).

Per-core algorithm (per batch b):
  - K: gpsimd cast-load fp32->bf16, then xbar DMA-transpose to d-major
    KT [d, n]. These two ops form a serial chain with each other and any
    other SWDGE DMA (a HW-deadlock guard serializes SWDGE DMAs against
    in-flight xbar transposes in issue order), so K is the ONLY tensor on
    that chain: ~27us/batch, comfortably under the PE's 41us/batch.
  - Q: gpsimd cast-load (natural layout), then transposed 128x128-tile-wise
    on the PE (transpose-mode matmul via identity, ~6us/batch) into PSUM,
    copied to SBUF by the Vector engine. Off the DMA chain entirely.
  - V: fp32 half-loads on the HWDGE scalar queue (async issue), cast to
    bf16 by the Vector engine; a ones-column is appended.
  - S^T[k, q] = sum_d KT[d,k] * QT[d,q] on PE (bf16, fp32 PSUM accum),
    k on partitions / q on free, 8 k-tiles x 2 q-halves x 6 d-chunks.
  - E = exp(S^T * (1/t)) on ScalarE (PSUM -> SBUF bf16), 1/t from input.
  - diagonal exclusion: E diag block *= (1 - I) mask.
  - out_psum[q, 0:769] = sum_k E^T[k,q] * [V | ones][k, :] on PE; col 768
    is the softmax denominator (ones-column trick).
  - out = out_psum[:, 0:768] * reciprocal(out_psum[:, 768]) -> bf16 SBUF
    -> HBM bf16 via scalar queue (host widens to fp32; ~0.2% rounding,
    well inside the 2e-2 gate).

Engine roles: GpSimd = K/Q cast-loads only. Sync = K transposes only.
Scalar = EXPs + V loads + output stores (all async or prompt). Vector =
V casts, QT copies, diag mask, reciprocal, output scale. Tags touched by
DMAs rotate per batch (DMA dep tracking is tag-coarse; ring reuse on one
tag creates false WAR edges).
"""

import os
import sys

if "/opt/trn_rl_repo" not in sys.path:
    sys.path.insert(0, "/opt/trn_rl_repo")

import numpy as np
import ml_dtypes

import concourse.bass as bass
import concourse.bacc as bacc
import concourse.tile as tile
from concourse import mybir
from concourse.bass_utils import run_bass_kernel_spmd

B, N, D = 64, 1024, 768
NCORES = 8
BPC = B // NCORES  # batches per core
P = 128
NT = N // P   # 8 n-tiles (also k-tiles / q-tiles)
DJ = D // P   # 6 d-chunks
H = NT // 2
F32 = mybir.dt.float32
BF16 = mybir.dt.bfloat16


def build_program(bpc: int = BPC) -> bacc.Bacc:
    nc = bacc.Bacc(
        "TRN2",
        target_bir_lowering=False,
        debug=False,
        num_devices=NCORES,
        num_swdge_queues=4,
    )
    q_h = nc.dram_tensor("q", [bpc, N, D], F32, kind="ExternalInput").ap()
    k_h = nc.dram_tensor("k", [bpc, N, D], F32, kind="ExternalInput").ap()
    v_h = nc.dram_tensor("v", [bpc, N, D], F32, kind="ExternalInput").ap()
    t_h = nc.dram_tensor("t", [1], F32, kind="ExternalInput").ap()
    m_h = nc.dram_tensor("mask", [P, P], BF16, kind="ExternalInput").ap()
    i_h = nc.dram_tensor("ident", [P, P], BF16, kind="ExternalInput").ap()
    o_h = nc.dram_tensor("o", [bpc, N, D], BF16, kind="ExternalOutput").ap()

    with tile.TileContext(nc) as tc:
        with (
            tc.tile_pool(name="const", bufs=1) as const,
            tc.tile_pool(name="stage", bufs=1) as stage,
            tc.tile_pool(name="vpool", bufs=1) as vpool,
            tc.tile_pool(name="tpose", bufs=1) as tpose,
            tc.tile_pool(name="epool", bufs=2) as epool,
            tc.tile_pool(name="opool", bufs=1) as opool,
            tc.tile_pool(name="small", bufs=8) as small,
            tc.tile_pool(name="ps_s", bufs=2, space="PSUM") as ps_s,
            tc.tile_pool(name="ps_o", bufs=2, space="PSUM") as ps_o,
            tc.tile_pool(name="ps_t", bufs=2, space="PSUM") as ps_t,
        ):
            # constants: 1/temperature broadcast, diag mask, identity
            t_bc = const.tile([P, 1], F32)
            nc.gpsimd.dma_start(out=t_bc, in_=t_h.to_broadcast((P, 1)))
            inv_t = const.tile([P, 1], F32)
            nc.vector.reciprocal(inv_t, t_bc)
            mask_sb = const.tile([P, P], BF16)
            nc.sync.dma_start(out=mask_sb, in_=m_h)
            ident = const.tile([P, P], BF16)
            nc.sync.dma_start(out=ident, in_=i_h)

            def load_batch(b):
                """Issue batch b's loads and K's xbar transpose.
                Returns (kT, qst, qT, v_sb); qT is filled later by
                emit_q_transposes."""
                kst = stage.tile([P, NT, D], BF16, tag=f"sk{b % 2}")
                qst = stage.tile([P, NT, D], BF16, tag=f"sq{b % 2}")
                # xbar 3D-out semantics: out[p, j, r] = in[r, j*128 + p],
                # j = (nt, dj) merged: kT[p,nt,dj,r] = K[nt*128+r, dj*128+p]
                kT = tpose.tile([P, NT, DJ, P], BF16, tag=f"tk{b % 2}")
                qT = tpose.tile([P, NT, DJ, P], BF16, tag=f"tq{b % 2}")
                if b == 0:
                    # K half 0 first (its transpose lands ~8us), then Q
                    # (feeds the PE transposes), then K half 1.
                    h0, h1 = slice(0, H), slice(H, NT)
                    r0, r1 = slice(0, H * P), slice(H * P, N)
                    nc.gpsimd.dma_start(
                        out=kst[:, h0, :],
                        in_=k_h[b, r0, :].rearrange("(nt p) d -> p nt d", p=P),
                    )
                    nc.sync.dma_start(
                        out=kT[:, h0, :, :], in_=kst[:, h0, :], transpose=True
                    )
                    nc.gpsimd.dma_start(
                        out=qst,
                        in_=q_h[b].rearrange("(nt p) d -> p nt d", p=P),
                    )
                    nc.gpsimd.dma_start(
                        out=kst[:, h1, :],
                        in_=k_h[b, r1, :].rearrange("(nt p) d -> p nt d", p=P),
                    )
                    nc.sync.dma_start(
                        out=kT[:, h1, :, :], in_=kst[:, h1, :], transpose=True
                    )
                else:
                    nc.gpsimd.dma_start(
                        out=kst,
                        in_=k_h[b].rearrange("(nt p) d -> p nt d", p=P),
                    )
                    nc.sync.dma_start(out=kT, in_=kst, transpose=True)
                    nc.gpsimd.dma_start(
                        out=qst,
                        in_=q_h[b].rearrange("(nt p) d -> p nt d", p=P),
                    )
                # V: fp32 half-loads on the scalar HWDGE queue, bf16 cast
                # on Vector. Two staging tags so the second issue never
                # blocks the scalar engine on the first half's cast.
                v_sb = vpool.tile([P, NT, D + 1], BF16, tag=f"v{b % 2}")
                for h in range(2):
                    v32 = stage.tile([P, H, D], F32, tag=f"v32{h}")
                    rows = slice(h * H * P, (h + 1) * H * P)
                    nc.scalar.dma_start(
                        out=v32,
                        in_=v_h[b, rows, :].rearrange("(nt p) d -> p nt d", p=P),
                    )
                    nc.vector.tensor_copy(
                        v_sb[:, h * H : (h + 1) * H, 0:D], v32
                    )
                nc.vector.memset(v_sb[:, :, D : D + 1], 1.0)
                return kT, qst, qT, v_sb

            def emit_q_transposes(batch, nts):
                """PE transpose-mode: qst [q, d] -> qT [d, q], one PSUM
                bank-tile (6 dj blocks) per n-tile, drained to SBUF by
                the Vector engine."""
                _, qst, qT, _ = batch
                for nt in nts:
                    pt = ps_t.tile([P, DJ, P], BF16, tag="pt")
                    for dj in range(DJ):
                        nc.tensor.transpose(
                            pt[:, dj, :],
                            qst[:, nt, dj * P : (dj + 1) * P],
                            ident,
                        )
                    nc.vector.tensor_copy(qT[:, nt, :, :], pt)

            pending = load_batch(0)
            emit_q_transposes(pending, range(NT))
            for b in range(bpc):
                kT, qst, qT, v_sb = pending
                if b + 1 < bpc:
                    pending = load_batch(b + 1)

                # ---- S^T = K Q^T (k on partitions), exp, diag-mask
                ev = epool.tile([P, NT, N], BF16, tag="ev")
                for kh in range(2):
                    for half in range(2):
                        for kt in range(4 * kh, 4 * kh + 4):
                            sT = ps_s.tile([P, 512], F32, tag="sT")
                            for dj in range(DJ):
                                nc.tensor.matmul(
                                    sT,
                                    lhsT=kT[:, kt, dj, :],
                                    rhs=qT[:, 4 * half : 4 * half + 4, dj, :],
                                    start=(dj == 0),
                                    stop=(dj == DJ - 1),
                                )
                            nc.scalar.activation(
                                ev[:, kt, half * 512 : half * 512 + 512],
                                sT,
                                mybir.ActivationFunctionType.Exp,
                                scale=inv_t,
                            )
                            if kt // 4 == half:
                                nc.vector.tensor_mul(
                                    ev[:, kt, kt * P : (kt + 1) * P],
                                    ev[:, kt, kt * P : (kt + 1) * P],
                                    mask_sb,
                                )

                # ---- out = (E^T @ [V | 1]) then normalize by ones-column.
                # Next batch's Q PE-transposes are interleaved after AV
                # q-tiles 1..4 so their PSUM-drain waits hide behind the
                # 3.3us AV groups (and batch b+1's Q load has landed by
                # qt1). Outputs staged four q-tiles per store (786 KB bf16
                # DMAs) on the scalar queue.
                o_sb = None
                for qt in range(NT):
                    if b + 1 < bpc and 1 <= qt <= 4:
                        emit_q_transposes(
                            pending, range(2 * (qt - 1), 2 * qt)
                        )
                    o_ps = ps_o.tile([P, D + 1], F32, tag="o_ps")
                    for kt in range(NT):
                        nc.tensor.matmul(
                            o_ps[:, 0:512],
                            lhsT=ev[:, kt, qt * P : (qt + 1) * P],
                            rhs=v_sb[:, kt, 0:512],
                            start=(kt == 0),
                            stop=(kt == NT - 1),
                        )
                    for kt in range(NT):
                        nc.tensor.matmul(
                            o_ps[:, 512 : D + 1],
                            lhsT=ev[:, kt, qt * P : (qt + 1) * P],
                            rhs=v_sb[:, kt, 512 : D + 1],
                            start=(kt == 0),
                            stop=(kt == NT - 1),
                        )
                    rs = small.tile([P, 1], F32, tag="rs")
                    nc.vector.reciprocal(rs, o_ps[:, D : D + 1])
                    if qt % 4 == 0:
                        o_sb = opool.tile(
                            [P, 4, D], BF16, tag=f"o{(2 * b + qt // 4) % 3}"
                        )
                    nc.vector.tensor_scalar_mul(
                        o_sb[:, qt % 4, :], o_ps[:, 0:D], rs
                    )
                    if qt % 4 == 3:
                        nc.scalar.dma_start(
                            out=o_h[b, (qt - 3) * P : (qt + 1) * P, :].rearrange(
                                "(j p) d -> p j d", p=P
                            ),
                            in_=o_sb,
                        )
    nc.finalize()
    return nc


_prog_cache: dict[int, bacc.Bacc] = {}


def _get_program(bpc: int) -> bacc.Bacc:
    if bpc not in _prog_cache:
        _prog_cache[bpc] = build_program(bpc)
    return _prog_cache[bpc]


def _run(Q, K, V, temperature, bpc: int = BPC, trace: bool = False):
    nc = _get_program(bpc)
    mask = (1.0 - np.eye(P, dtype=np.float32)).astype(ml_dtypes.bfloat16)
    ident = np.eye(P, dtype=np.float32).astype(ml_dtypes.bfloat16)
    t = np.asarray(temperature, dtype=np.float32).reshape(1)
    in_maps = []
    for c in range(NCORES):
        sl = slice(c * bpc, (c + 1) * bpc)
        in_maps.append(
            {
                "q": np.ascontiguousarray(Q[sl], dtype=np.float32),
                "k": np.ascontiguousarray(K[sl], dtype=np.float32),
                "v": np.ascontiguousarray(V[sl], dtype=np.float32),
                "t": t,
                "mask": mask,
                "ident": ident,
            }
        )
    res = run_bass_kernel_spmd(
        nc, in_maps, core_ids=list(range(NCORES)), trace=trace
    )
    out = np.concatenate([r["o"] for r in res.results], axis=0)
    return out, res


def kernel(Q, K, V, temperature):
    # If BASS_TRACE leaked into the environment, the trace path would need
    # antenv.axon_hooks (absent in this image) and crash; force it off for
    # the plain grading path.
    if os.environ.get("BASS_TRACE"):
        try:
            import antenv.axon_hooks  # noqa: F401
        except ImportError:
            os.environ.pop("BASS_TRACE", None)
    out, _ = _run(Q, K, V, temperature)
    return np.asarray(out).astype(np.float32)
